# revision 48
# baseline (speedup 1.0000x reference)
"""Trainium2 Bass kernel for NormalAttention (embedded gaussian, non-local block).

Reference computation per batch sample b (B=8, C=256, Ck=64, N=48*48=2304):
    q = Wq @ x + bq            (64, N)
    k = Wk @ x + bk            (64, N)
    e[i,j] = q[:,i] . k[:,j]   (N, N)
    E = exp(e);  E[i,j] /= sum_j E[i,j]
    v = Wv @ x + bv            (256, N)
    att[c,j] = sum_i v[c,i] * E[i,j]
    out = Wg @ att + bg        (256, N)

Sharding: pure data parallel, one batch sample per NeuronCore (8 cores).

Per-core kernel structure (cost-model timed at ~73.1us/core, from the
82.0us baseline):
  - all matmuls in bf16; the gamma 1x1 conv is folded into the V projection
    on the host (W_comb = (Wg@Wv)^T, bvg = Wg@bv); all weights + biases
    arrive in one small bf16 blob ahead of the x pieces (the serial
    HWDGE/DMA pipeline costs ~0.63us per transfer, so DMA count and order
    matter; bias columns are up-cast to f32 on DVE once since tensor_scalar
    ops require f32 scalars).
  - phase 1 is ACT-paced at ~2.48us/chunk: per 128-row i-chunk, energy goes
    into a (128,1280)+(128,1024) PSUM ping-pong (5 banks); exp on ACT with
    accum_out row-summing the 1280 half; DVE reduces the 1024 half and folds
    1/s into vt.
  - the remaining 3 PSUM banks run ROTATING pass-2 accumulator segments
    (BANK_PLAN): each bank hosts a sequence of (group, i-range) partial
    accumulations; era-1 segments dump their partial to SBUF (DVE copy,
    normal priority - the dump gates the next segment's burst) and the
    remainder is re-accumulated in phase 2 and combined with the partial
    via scalar_tensor_tensor.  This fills PE's phase-1 idle with pass-2
    work: pass-2 leftover after phase 1 is ~16.3us (vs 23.3 baseline).
  - the two 256-wide output tiles (j 1024:1280) share one PSUM bank as a
    PAIR.  Hardware matmul start=True resets the WHOLE bank (CoreSim only
    resets the addressed region!), so the pair bank is pre-zeroed by DVE
    and every pair matmul accumulates.
  - V^T projection fills head/early-phase-1 PE slots (demoted priority).
    Its psum slots share the rotation banks; bank2 (which seeds the PAIR
    chain) gets the earliest V chunks so every bank's first segment is
    released as early as possible.  GPSIMD cannot touch PSUM on real HW,
    so all PSUM-side bias/cast ops stay on DVE/ACT.
  - phase-2 finish order puts the smallest chains last; the final group's
    columns are split across two PSUM banks so its first half's store
    chain (DVE bias -> DGE -> DMA -> sem, ~3.5us of fixed latency)
    overlaps the second half's matmuls.
"""

import os
import sys

sys.path.insert(0, "/opt/trn_rl_repo")

os.environ["BASS_NEVER_TRACE"] = "1"

_jp = os.environ.get("JAX_PLATFORMS")
if _jp and "axon" not in _jp and "jax" not in sys.modules:
    os.environ.pop("JAX_PLATFORMS", None)

import numpy as np
import ml_dtypes

import concourse.bass as bass
import concourse.mybir as mybir
import concourse.tile as tile
from concourse import bacc
from concourse.bass_utils import run_bass_kernel_spmd

B, C, CK, H, W = 8, 256, 64, 48, 48
N = H * W            # 2304
P = 128
NI = N // P          # 18 i-chunks
NCORES = 8

BF16 = mybir.dt.bfloat16
F32 = mybir.dt.float32
AF = mybir.ActivationFunctionType
ALU = mybir.AluOpType
AX = mybir.AxisListType

# energy ping-pong PSUM split: (128,1280)=3 banks + (128,1024)=2 banks.
EA, EB = 1280, 1024
E_SPLITS = [
    (0, EA, "engA", [(0, 512), (512, 512), (1024, 256)]),
    (EA, EB, "engB", [(0, 512), (512, 512)]),
]
# QK-projection j-tiles (must not cross the EA boundary for energy subs)
J_TILES = [(0, 512), (512, 512), (1024, 256), (1280, 512), (1792, 512)]

# pass-2 groups: (name, ch, j0, jw).  G8/G9 are the two 256-wide tiles and
# share one PSUM bank slot (PAIR).
GROUPS = {
    "G0": (0, 0, 512), "G1": (1, 0, 512),
    "G2": (0, 512, 512), "G3": (1, 512, 512),
    "G4": (0, 1280, 512), "G5": (1, 1280, 512),
    "G6": (0, 1792, 512), "G7": (1, 1792, 512),
    "G8": (0, 1024, 256), "G9": (1, 1024, 256),
}

# rotation plan: per bank, segments (group, i_hi, dump).  A segment
# accumulates i = 0..i_hi during phase 1, then either dumps its partial to
# SBUF (freeing the bank) or stays live into phase 2.  "PAIR" runs G8 in
# psum cols 0:256 and G9 in 256:512.
BANK_PLAN = [
    [("G0", 0, 6, True), ("G3", 0, 12, True), ("G6", 0, 14, False)],
    [("G1", 0, 6, True), ("G4", 0, 12, True), ("G7", 0, 14, False)],
    [("G2", 0, 6, True), ("G5", 0, 12, True), ("PAIR", 0, 14, False)],
]

N_WARM = 7           # PE warmup matmuls issued under the input DMAs


def _build_nc():
    nc = bacc.Bacc("TRN2", target_bir_lowering=False, debug=False,
                   num_devices=NCORES)

    x_d = nc.dram_tensor("x", [2, P, N], BF16, kind="ExternalInput")
    wblob_d = nc.dram_tensor("wblob", [P, 2 * P + C + 4], BF16,
                             kind="ExternalInput")
    wrest_d = nc.dram_tensor("wrest", [P, 2 * C], BF16, kind="ExternalInput")
    out_d = nc.dram_tensor("out", [2, P, N], F32, kind="ExternalOutput")
    warm_d = nc.dram_tensor("warm", [P, 1], F32, kind="ExternalOutput")

    with tile.TileContext(nc) as tc:
        with (
            tc.tile_pool(name="consts", bufs=1) as consts,
            tc.tile_pool(name="big", bufs=1) as big,
            tc.tile_pool(name="work", bufs=6) as work,
            tc.tile_pool(name="ps_big", bufs=1, space="PSUM") as ps_big,
            tc.tile_pool(name="ps_rot", bufs=1, space="PSUM") as ps_rot,
        ):
            # ---------------- PE warmup under the input DMAs ----------------
            dummy = consts.tile([P, 512], BF16)
            nc.gpsimd.memset(dummy[:], 0)
            warm_sb = consts.tile([P, 1], F32)
            # 1-element exp: forces the implicit ACT_TABLE_LOAD (~1.3us) to
            # run at t~0 under the DMAs instead of gating the first exp
            nc.scalar.activation(warm_sb[0:1, 0:1], dummy[0:1, 0:1], AF.Exp)
            psd = ps_rot.tile([P, 512], F32, tag="rot0")
            for w in range(N_WARM):
                nc.tensor.matmul(psd[:], dummy[:, :P], dummy[:],
                                 start=(w == 0), stop=(w == N_WARM - 1))
            nc.vector.tensor_copy(warm_sb, psd[:, 0:1])
            nc.scalar.dma_start(warm_d[:], warm_sb)

            # ---------------- inputs ----------------
            # x pieces on the SP queue; small weight/bias blobs on the ACT
            # queue so they land in parallel with the first x piece.
            xt = big.tile([P, 2, N], BF16)
            wblob = consts.tile([P, 2 * P + C + 4], BF16)
            wrest = consts.tile([P, 2 * C], BF16)
            x_r = x_d[:].rearrange("c p n -> p c n")
            nc.sync.dma_start(xt[:, :, 0:512], x_r[:, :, 0:512])
            nc.sync.dma_start(wblob[:], wblob_d[:])
            nc.sync.dma_start(xt[:, :, 512:1024], x_r[:, :, 512:1024])
            nc.sync.dma_start(xt[:, :, 1024:1280], x_r[:, :, 1024:1280])
            nc.sync.dma_start(xt[:, :, 1280:1792], x_r[:, :, 1280:1792])
            nc.sync.dma_start(xt[:, :, 1792:N], x_r[:, :, 1792:N])
            nc.sync.dma_start(wrest[:], wrest_d[:])

            def wv(c):
                return wrest[:, c * C:(c + 1) * C]

            wqk = wblob[:, 0:2 * P]
            vb = wblob[:, 2 * P + 4:]               # bvg broadcast
            # scalar (per-partition) bias operands must be f32: one tiny
            # up-cast of the 4 bias columns right after wblob lands
            fbias = consts.tile([P, 4], F32)
            nc.vector.tensor_copy(fbias[:], wblob[:, 2 * P:2 * P + 4])
            qb = fbias[0:CK, 0:1]
            kb = fbias[0:CK, 1:2]
            gbias = fbias[:, 2:4]                   # gamma bias, col per ch

            # ---------------- Q / K projections ----------------
            q_t = big.tile([CK, N], BF16)
            k_t = big.tile([CK, N], BF16)

            def _ps_tile(tag, name):
                pool = ps_big if tag in ("engA", "engB") else ps_rot
                return pool.tile([P, 512], F32, tag=tag, name=name)

            def psk_mms(j0, jw, tag, on_act=False):
                psk = _ps_tile(tag, "psk")
                for c in range(2):
                    nc.tensor.matmul(psk[:CK, :jw],
                                     wqk[:, c * P + CK:(c + 1) * P],
                                     xt[:, c, j0:j0 + jw],
                                     start=(c == 0), stop=(c == 1))
                if on_act:
                    nc.scalar.activation(k_t[:, j0:j0 + jw], psk[:CK, :jw],
                                         AF.Identity, bias=kb)
                else:
                    nc.vector.tensor_scalar_add(k_t[:, j0:j0 + jw],
                                                psk[:CK, :jw], kb)

            def psq_mms(j0, jw, tag, defer=False):
                psq = _ps_tile(tag, "psq")
                for c in range(2):
                    nc.tensor.matmul(psq[:CK, :jw], wqk[:, c * P:c * P + CK],
                                     xt[:, c, j0:j0 + jw],
                                     start=(c == 0), stop=(c == 1))
                if defer:
                    # q-tiles 1..4 are not needed until energy chunk 4+;
                    # demote so DVE handles critical-path work first
                    with tc.high_priority(offset=-30000):
                        nc.vector.tensor_scalar_add(q_t[:, j0:j0 + jw],
                                                    psq[:CK, :jw], qb)
                else:
                    nc.vector.tensor_scalar_add(q_t[:, j0:j0 + jw],
                                                psq[:CK, :jw], qb)

            # shared big SBUF tensors
            vt = big.tile([P, NI, C], BF16)       # V^T, later scaled by 1/s
            expA = big.tile([P, NI, EA], BF16)
            expB = big.tile([P, NI, EB], BF16)
            s_half = big.tile([P, NI, 2], F32)
            invs = big.tile([P, NI], F32)

            eps_of = {}

            def emit_energy(kk, part):
                (base, width, tag, subs) = E_SPLITS[part]
                eps = ps_big.tile([P, width], F32, tag=tag, name=f"eps{part}")
                for (o0, ow) in subs:
                    nc.tensor.matmul(
                        eps[:, o0:o0 + ow],
                        q_t[:, kk * P:(kk + 1) * P],
                        k_t[:, base + o0:base + o0 + ow],
                        start=True, stop=True)
                eps_of.setdefault(kk, [None, None])[part] = eps

            # head: k_t tiles 0..2 + q_t tile 0 unlock energy(0) A; ACT
            # (idle until the first exp) takes k0/k2, DVE takes q0/k1.
            # Only q_t[:, 0:128] gates energy(0), so its bias lands in a
            # tiny op and the rest of tile 0 is deferred.
            psk_mms(*J_TILES[0], tag="engA", on_act=True)
            psq0 = _ps_tile("engB", "psq0")
            for c in range(2):
                nc.tensor.matmul(psq0[:CK, :512], wqk[:, c * P:c * P + CK],
                                 xt[:, c, 0:512],
                                 start=(c == 0), stop=(c == 1))
            nc.vector.tensor_scalar_add(q_t[:, 0:P], psq0[:CK, 0:P], qb)
            nc.vector.tensor_scalar_add(q_t[:, P:512], psq0[:CK, P:512], qb)
            psk_mms(*J_TILES[1], tag="rot0")
            psk_mms(*J_TILES[2], tag="rot1", on_act=True)
            emit_energy(0, 0)
            psk_mms(*J_TILES[3], tag="rot1")
            psk_mms(*J_TILES[4], tag="rot2")
            emit_energy(0, 1)
            psq_mms(*J_TILES[1], tag="rot0", defer=True)
            psq_mms(*J_TILES[2], tag="rot1", defer=True)
            psq_mms(*J_TILES[3], tag="rot2", defer=True)
            psq_mms(*J_TILES[4], tag="rot0", defer=True)

            # ---------------- V^T projection ----------------
            # fills PE idle in the head and early phase-1.  Slots are
            # assigned sequentially (bank0 gets chunks 0-5, ...) so each
            # rotation bank's first pass-2 segment is released as early as
            # possible; the first banks' bias-adds run on DVE (fast, early),
            # the rest on the otherwise-idle GPSIMD engine.
            with tc.high_priority(offset=-100000):
                for i in range(NI):
                    psv = ps_rot.tile([P, 512], F32, tag=f"rot{2 - i // 6}",
                                      name="psv")
                    for c in range(2):
                        nc.tensor.matmul(psv[:, :C],
                                         xt[:, c, i * P:(i + 1) * P],
                                         wv(c), start=(c == 0), stop=(c == 1))
                    nc.vector.tensor_tensor(vt[:, i], psv[:, :C], vb,
                                            ALU.add)

            # ---------------- pass 1 pipeline ----------------
            for k in range(NI):
                nc.scalar.activation(
                    out=expA[:, k, :], in_=eps_of[k][0][:],
                    func=AF.Exp, accum_out=s_half[:, k, 0:1])
                nc.scalar.activation(
                    out=expB[:, k, :], in_=eps_of[k][1][:],
                    func=AF.Exp)
                nc.vector.tensor_reduce(
                    s_half[:, k, 1:2], expB[:, k, :],
                    axis=AX.X, op=ALU.add)
                nc.vector.tensor_tensor(invs[:, k:k + 1], s_half[:, k, 0:1],
                                        s_half[:, k, 1:2], ALU.add)
                nc.vector.reciprocal(invs[:, k:k + 1], invs[:, k:k + 1])
                nc.vector.tensor_scalar_mul(vt[:, k], vt[:, k],
                                            invs[:, k:k + 1])
                if k + 1 < NI:
                    emit_energy(k + 1, 0)
                    emit_energy(k + 1, 1)

            # ---------------- rotating pass-2 segments ----------------
            def exp_slice(i, j0, jw):
                if j0 + jw <= EA:
                    return expA[:, i, j0:j0 + jw]
                return expB[:, i, j0 - EA:j0 - EA + jw]

            def pair_mms(seg, i, start):
                # the two 256-wide regions share one PSUM bank; hardware
                # 'start' resets the whole bank, so the bank is pre-zeroed
                # once and every matmul accumulates (start=False)
                if start:
                    nc.vector.memset(seg[:], 0)
                for gi, g in enumerate(("G8", "G9")):
                    (ch, j0, jw) = GROUPS[g]
                    nc.tensor.matmul(
                        seg[:, gi * 256:gi * 256 + 256],
                        vt[:, i, ch * P:(ch + 1) * P],
                        exp_slice(i, j0, jw),
                        start=False, stop=False, skip_group_check=True)

            def group_mms(seg, g, i, start):
                if g == "PAIR":
                    pair_mms(seg, i, start)
                    return
                (ch, j0, jw) = GROUPS[g]
                nc.tensor.matmul(
                    seg[:, :jw],
                    vt[:, i, ch * P:(ch + 1) * P],
                    exp_slice(i, j0, jw),
                    start=start, stop=False, skip_group_check=True)

            partials = {}
            live_seg = {}
            for b, plan in enumerate(BANK_PLAN):
                for (g, i_lo, i_hi, dumped) in plan:
                    seg = ps_rot.tile([P, 512], F32, tag=f"rot{b}",
                                      name=f"seg_{g}")
                    for i in range(i_lo, i_hi + 1):
                        group_mms(seg, g, i, start=(i == i_lo))
                    if dumped:
                        # dump at normal priority: it gates the next
                        # segment's burst, so DVE must not defer it
                        part = big.tile([P, 512], F32, name=f"part_{g}")
                        nc.vector.tensor_copy(part[:], seg[:])
                        partials[g] = (part, i_lo, i_hi)
                    else:
                        live_seg[g] = (seg, i_lo, i_hi)

            # ---------------- phase 2 ----------------
            _queue_flip = [0]

            def emit_sbuf_store(g, ap):
                (ch, j0, jw) = GROUPS[g]
                _queue_flip[0] ^= 1
                eng = nc.sync if _queue_flip[0] else nc.scalar
                eng.dma_start(out_d[ch, :, j0:j0 + jw], ap)

            # live groups finish in place; bias-add, then store
            def finish_live(g):
                seg, i_lo, i_hi = live_seg[g]
                if g == "PAIR":
                    raise AssertionError("use finish_pair_region")
                else:
                    (ch, j0, jw) = GROUPS[g]
                    for i in range(i_hi + 1, NI - 1):
                        group_mms(seg, g, i, start=False)
                    nc.tensor.matmul(
                        seg[:, :jw],
                        vt[:, NI - 1, ch * P:(ch + 1) * P],
                        exp_slice(NI - 1, j0, jw),
                        start=False, stop=True, skip_group_check=True)
                    ot = work.tile([P, 512], F32, tag="out")
                    nc.vector.tensor_scalar_add(ot[:, :jw], seg[:, :jw],
                                                gbias[:, ch:ch + 1])
                    emit_sbuf_store(g, ot[:, :jw])

            # the PAIR's two regions close independently: G9 early in
            # phase 2, G8 last (its short chain is the kernel tail)
            def finish_pair_joint():
                seg, i_lo, i_hi = live_seg["PAIR"]
                for i in range(i_hi + 1, NI - 1):
                    pair_mms(seg, i, start=False)
                for gi, gg in enumerate(("G8", "G9")):
                    (ch, j0, jw) = GROUPS[gg]
                    nc.tensor.matmul(
                        seg[:, gi * 256:gi * 256 + 256],
                        vt[:, NI - 1, ch * P:(ch + 1) * P],
                        exp_slice(NI - 1, j0, jw),
                        start=False, stop=True, skip_group_check=True)
                for gi, gg in enumerate(("G8", "G9")):
                    (ch, j0, jw) = GROUPS[gg]
                    ot = work.tile([P, 256], F32, tag="out2")
                    nc.vector.tensor_scalar_add(
                        ot[:], seg[:, gi * 256:gi * 256 + 256],
                        gbias[:, ch:ch + 1])
                    eng = nc.sync if gi == 0 else nc.scalar
                    eng.dma_start(out_d[ch, :, j0:j0 + jw], ot[:])

            # dumped groups re-accumulate their remaining i-range in a fresh
            # slot, then combine (psum + gbias) + partial on DVE
            def finish_dumped(g, tag, pool, half=None, cols=None):
                part, i_lo, i_hi = partials[g]
                (ch, j0, jw) = GROUPS[g]
                if cols is not None:
                    c0, cw = cols
                else:
                    c0, cw = (0, jw) if half is None else (half * (jw // 2),
                                                          jw // 2)
                seg = pool.tile([P, 512], F32, tag=tag, name=f"fin_{g}")
                todo = list(range(0, i_lo)) + list(range(i_hi + 1, NI))
                for n, i in enumerate(todo):
                    nc.tensor.matmul(
                        seg[:, :cw],
                        vt[:, i, ch * P:(ch + 1) * P],
                        exp_slice(i, j0 + c0, cw),
                        start=(n == 0), stop=(n == len(todo) - 1),
                        skip_group_check=True)
                ot = work.tile([P, 512], F32, tag="out")
                nc.vector.scalar_tensor_tensor(
                    ot[:, :cw], seg[:, :cw], gbias[:, ch:ch + 1],
                    part[:, c0:c0 + cw], ALU.add, ALU.add)
                (ch_, j0_, jw_) = GROUPS[g]
                _queue_flip[0] ^= 1
                eng = nc.sync if _queue_flip[0] else nc.scalar
                eng.dma_start(out_d[ch_, :, j0_ + c0:j0_ + c0 + cw],
                              ot[:, :cw])

            # order: live 512 groups first (their stores overlap the rest),
            # dumped groups through freed energy + rot banks, PAIR last
            # (small, direct-from-PSUM stores).
            finish_live("G6")
            finish_live("G7")
            finish_dumped("G0", "engA", ps_big)
            finish_dumped("G1", "engB", ps_big)
            finish_dumped("G3", "rot0", ps_rot)
            finish_dumped("G4", "rot1", ps_rot)
            finish_dumped("G5", "engA", ps_big)
            finish_pair_joint()
            # last group: columns split across two banks so the first
            # half's store chain overlaps the second half's matmuls
            finish_dumped("G2", "engB", ps_big, cols=(0, 256))
            finish_dumped("G2", "engA", ps_big, cols=(256, 256))

    nc.compile()
    return nc


_NC_CACHE = []


def _get_nc():
    if not _NC_CACHE:
        _NC_CACHE.append(_build_nc())
    return _NC_CACHE[0]


def _prep_inputs(x, query_weight, query_bias, key_weight, key_bias,
                 value_weight, value_bias, gamma_weight, gamma_bias):
    bf16 = ml_dtypes.bfloat16
    x = np.asarray(x, np.float32).reshape(B, C, N)
    qw = np.asarray(query_weight, np.float32)[:, :, 0, 0]   # (64, 256)
    kw = np.asarray(key_weight, np.float32)[:, :, 0, 0]     # (64, 256)
    vw = np.asarray(value_weight, np.float32)[:, :, 0, 0]   # (256, 256)
    gw = np.asarray(gamma_weight, np.float32)[:, :, 0, 0]   # (256, 256)

    # wqk[p, c*128+m] = W_cat^T[c*128+p, m]  (W_cat = [Wq; Wk], (128, 256))
    wcat_t = np.concatenate([qw, kw], axis=0).T              # (256, 128)
    wqk = np.ascontiguousarray(
        wcat_t.reshape(2, P, P).transpose(1, 0, 2).reshape(P, 2 * P))

    # gamma folds into the V projection: W_comb = (Wg @ Wv)^T, bvg = Wg @ bv
    w_comb = (gw @ vw).T                                    # (c_in, o)
    wrest = np.ascontiguousarray(
        w_comb.reshape(2, P, C).transpose(1, 0, 2).reshape(P, 2 * C))
    bvg = gw @ np.asarray(value_bias, np.float32)

    wblob = np.zeros((P, 2 * P + C + 4), np.float32)
    wblob[:, 0:2 * P] = wqk
    wblob[0:CK, 2 * P] = np.asarray(query_bias, np.float32)
    wblob[0:CK, 2 * P + 1] = np.asarray(key_bias, np.float32)
    wblob[:, 2 * P + 2:2 * P + 4] = \
        np.asarray(gamma_bias, np.float32).reshape(2, P).T
    wblob[:, 2 * P + 4:] = bvg[None, :]

    base = {
        "wblob": wblob.astype(bf16),
        "wrest": wrest.astype(bf16),
    }
    in_maps = []
    for b in range(B):
        m = dict(base)
        m["x"] = x[b].reshape(2, P, N).astype(bf16)
        in_maps.append(m)
    return in_maps


def kernel(x, query_weight, query_bias, key_weight, key_bias,
           value_weight, value_bias, gamma_weight, gamma_bias, k):
    assert int(k) == C // CK
    in_maps = _prep_inputs(x, query_weight, query_bias, key_weight, key_bias,
                           value_weight, value_bias, gamma_weight, gamma_bias)
    nc = _get_nc()
    res = run_bass_kernel_spmd(nc, in_maps, core_ids=list(range(NCORES)))

    out = np.empty((B, C, H, W), np.float32)
    for b in range(B):
        out[b] = res.results[b]["out"].reshape(C, H, W)
    return out


# revision 80
# speedup vs baseline: 1.0057x; 1.0057x over previous
"""Trainium2 Bass kernel for NormalAttention (embedded gaussian, non-local block).

Reference computation per batch sample b (B=8, C=256, Ck=64, N=48*48=2304):
    q = Wq @ x + bq            (64, N)
    k = Wk @ x + bk            (64, N)
    e[i,j] = q[:,i] . k[:,j]   (N, N)
    E = exp(e);  E[i,j] /= sum_j E[i,j]
    v = Wv @ x + bv            (256, N)
    att[c,j] = sum_i v[c,i] * E[i,j]
    out = Wg @ att + bg        (256, N)

Sharding: pure data parallel, one batch sample per NeuronCore (8 cores).

Per-core kernel structure (cost-model timed at ~72.6us/core, from the
82.0us baseline):
  - all matmuls in bf16; the gamma 1x1 conv is folded into the V projection
    on the host (W_comb = (Wg@Wv)^T, bvg = Wg@bv); all weights + biases
    arrive in one small bf16 blob ahead of the x pieces (the serial
    HWDGE/DMA pipeline costs ~0.63us per transfer, so DMA count and order
    matter; bias columns are up-cast to f32 on DVE once since tensor_scalar
    ops require f32 scalars).
  - phase 1 is ACT-paced at ~2.48us/chunk: per 128-row i-chunk, energy goes
    into a (128,1280)+(128,1024) PSUM ping-pong (5 banks); exp on ACT with
    accum_out row-summing the 1280 half; DVE reduces the 1024 half and folds
    1/s into vt.
  - the remaining 3 PSUM banks run ROTATING pass-2 accumulator segments
    (BANK_PLAN): each bank hosts a sequence of (group, i-range) partial
    accumulations; era-1 segments dump their partial to SBUF (DVE copy,
    normal priority - the dump gates the next segment's burst) and the
    remainder is re-accumulated in phase 2 and combined with the partial
    via scalar_tensor_tensor.  This fills PE's phase-1 idle with pass-2
    work: pass-2 leftover after phase 1 is ~16.1us (vs 23.3 baseline).
  - the two 256-wide output tiles (j 1024:1280) share one PSUM bank as a
    PAIR.  Hardware matmul start=True resets the WHOLE bank (CoreSim only
    resets the addressed region!), so the pair bank is pre-zeroed by DVE
    and every pair matmul accumulates.
  - V^T projection fills head/early-phase-1 PE slots (demoted priority).
    Its psum slots share the rotation banks; bank2 (which seeds the PAIR
    chain) gets the earliest V chunks so every bank's first segment is
    released as early as possible.  GPSIMD cannot touch PSUM on real HW,
    so all PSUM-side bias/cast ops stay on DVE/ACT.
  - phase-2 finish order puts the smallest chains last; the final group's
    columns are split across two PSUM banks so its first half's store
    chain (DVE bias -> DGE -> DMA -> sem, ~3.5us of fixed latency)
    overlaps the second half's matmuls.
"""

import os
import sys

sys.path.insert(0, "/opt/trn_rl_repo")

os.environ["BASS_NEVER_TRACE"] = "1"

_jp = os.environ.get("JAX_PLATFORMS")
if _jp and "axon" not in _jp and "jax" not in sys.modules:
    os.environ.pop("JAX_PLATFORMS", None)

import numpy as np
import ml_dtypes

import concourse.bass as bass
import concourse.mybir as mybir
import concourse.tile as tile
from concourse import bacc
from concourse.bass_utils import run_bass_kernel_spmd

B, C, CK, H, W = 8, 256, 64, 48, 48
N = H * W            # 2304
P = 128
NI = N // P          # 18 i-chunks
NCORES = 8

BF16 = mybir.dt.bfloat16
F32 = mybir.dt.float32
AF = mybir.ActivationFunctionType
ALU = mybir.AluOpType
AX = mybir.AxisListType

# energy ping-pong PSUM split: (128,1280)=3 banks + (128,1024)=2 banks.
EA, EB = 1280, 1024
E_SPLITS = [
    (0, EA, "engA", [(0, 512), (512, 512), (1024, 256)]),
    (EA, EB, "engB", [(0, 512), (512, 512)]),
]
# QK-projection j-tiles (must not cross the EA boundary for energy subs)
J_TILES = [(0, 512), (512, 512), (1024, 256), (1280, 512), (1792, 512)]

# pass-2 groups: (name, ch, j0, jw).  G8/G9 are the two 256-wide tiles and
# share one PSUM bank slot (PAIR).
GROUPS = {
    "G0": (0, 0, 512), "G1": (1, 0, 512),
    "G2": (0, 512, 512), "G3": (1, 512, 512),
    "G4": (0, 1280, 512), "G5": (1, 1280, 512),
    "G6": (0, 1792, 512), "G7": (1, 1792, 512),
    "G8": (0, 1024, 256), "G9": (1, 1024, 256),
}

# rotation plan: per bank, segments (group, i_hi, dump).  A segment
# accumulates i = 0..i_hi during phase 1, then either dumps its partial to
# SBUF (freeing the bank) or stays live into phase 2.  "PAIR" runs G8 in
# psum cols 0:256 and G9 in 256:512.
BANK_PLAN = [
    [("G0", 0, 6, True), ("G3", 0, 13, True), ("G6", 0, 14, False)],
    [("G1", 0, 6, True), ("G4", 0, 13, True), ("G7", 0, 14, False)],
    [("G2", 0, 6, True), ("G5", 0, 13, True), ("PAIR", 0, 14, False)],
]

N_WARM = 6           # PE warmup matmuls issued under the input DMAs


def _build_nc():
    nc = bacc.Bacc("TRN2", target_bir_lowering=False, debug=False,
                   num_devices=NCORES)

    x_d = nc.dram_tensor("x", [2, P, N], BF16, kind="ExternalInput")
    wblob_d = nc.dram_tensor("wblob", [P, 2 * P + C + 4], BF16,
                             kind="ExternalInput")
    wrest_d = nc.dram_tensor("wrest", [P, 2 * C], BF16, kind="ExternalInput")
    out_d = nc.dram_tensor("out", [2, P, N], F32, kind="ExternalOutput")
    warm_d = nc.dram_tensor("warm", [P, 1], F32, kind="ExternalOutput")

    with tile.TileContext(nc) as tc:
        with (
            tc.tile_pool(name="consts", bufs=1) as consts,
            tc.tile_pool(name="big", bufs=1) as big,
            tc.tile_pool(name="work", bufs=6) as work,
            tc.tile_pool(name="ps_big", bufs=1, space="PSUM") as ps_big,
            tc.tile_pool(name="ps_rot", bufs=1, space="PSUM") as ps_rot,
        ):
            # ---------------- PE warmup under the input DMAs ----------------
            dummy = consts.tile([P, 512], BF16)
            nc.gpsimd.memset(dummy[:], 0)
            warm_sb = consts.tile([P, 1], F32)
            # 1-element exp: forces the implicit ACT_TABLE_LOAD (~1.3us) to
            # run at t~0 under the DMAs instead of gating the first exp
            nc.scalar.activation(warm_sb[0:1, 0:1], dummy[0:1, 0:1], AF.Exp)
            psd = ps_rot.tile([P, 512], F32, tag="rot0")
            for w in range(N_WARM):
                nc.tensor.matmul(psd[:], dummy[:, :P], dummy[:],
                                 start=(w == 0), stop=(w == N_WARM - 1))
            nc.vector.tensor_copy(warm_sb, psd[:, 0:1])
            nc.scalar.dma_start(warm_d[:], warm_sb)

            # ---------------- inputs ----------------
            # x pieces on the SP queue; small weight/bias blobs on the ACT
            # queue so they land in parallel with the first x piece.
            xt = big.tile([P, 2, N], BF16)
            wblob = consts.tile([P, 2 * P + C + 4], BF16)
            wrest = consts.tile([P, 2 * C], BF16)
            x_r = x_d[:].rearrange("c p n -> p c n")
            nc.sync.dma_start(xt[:, :, 0:512], x_r[:, :, 0:512])
            nc.sync.dma_start(wblob[:], wblob_d[:])
            nc.sync.dma_start(xt[:, :, 512:1280], x_r[:, :, 512:1280])
            nc.sync.dma_start(xt[:, :, 1280:1792], x_r[:, :, 1280:1792])
            nc.sync.dma_start(xt[:, :, 1792:N], x_r[:, :, 1792:N])
            nc.sync.dma_start(wrest[:], wrest_d[:])

            def wv(c):
                return wrest[:, c * C:(c + 1) * C]

            wqk = wblob[:, 0:2 * P]
            vb = wblob[:, 2 * P + 4:]               # bvg broadcast
            # scalar (per-partition) bias operands must be f32: one tiny
            # up-cast of the 4 bias columns right after wblob lands
            fbias = consts.tile([P, 4], F32)
            nc.vector.tensor_copy(fbias[:], wblob[:, 2 * P:2 * P + 4])
            qb = fbias[0:CK, 0:1]
            kb = fbias[0:CK, 1:2]
            gbias = fbias[:, 2:4]                   # gamma bias, col per ch

            # ---------------- Q / K projections ----------------
            q_t = big.tile([CK, N], BF16)
            k_t = big.tile([CK, N], BF16)

            def _ps_tile(tag, name):
                pool = ps_big if tag in ("engA", "engB") else ps_rot
                return pool.tile([P, 512], F32, tag=tag, name=name)

            def psk_mms(j0, jw, tag, on_act=False):
                psk = _ps_tile(tag, "psk")
                for c in range(2):
                    nc.tensor.matmul(psk[:CK, :jw],
                                     wqk[:, c * P + CK:(c + 1) * P],
                                     xt[:, c, j0:j0 + jw],
                                     start=(c == 0), stop=(c == 1))
                if on_act:
                    nc.scalar.activation(k_t[:, j0:j0 + jw], psk[:CK, :jw],
                                         AF.Identity, bias=kb)
                else:
                    nc.vector.tensor_scalar_add(k_t[:, j0:j0 + jw],
                                                psk[:CK, :jw], kb)

            def psq_mms(j0, jw, tag, defer=False):
                psq = _ps_tile(tag, "psq")
                for c in range(2):
                    nc.tensor.matmul(psq[:CK, :jw], wqk[:, c * P:c * P + CK],
                                     xt[:, c, j0:j0 + jw],
                                     start=(c == 0), stop=(c == 1))
                if defer:
                    # q-tiles 1..4 are not needed until energy chunk 4+;
                    # demote so DVE handles critical-path work first
                    with tc.high_priority(offset=-30000):
                        nc.vector.tensor_scalar_add(q_t[:, j0:j0 + jw],
                                                    psq[:CK, :jw], qb)
                else:
                    nc.vector.tensor_scalar_add(q_t[:, j0:j0 + jw],
                                                psq[:CK, :jw], qb)

            # shared big SBUF tensors
            vt = big.tile([P, NI, C], BF16)       # V^T, later scaled by 1/s
            expA = big.tile([P, NI, EA], BF16)
            expB = big.tile([P, NI, EB], BF16)
            s_half = big.tile([P, NI, 2], F32)
            invs = big.tile([P, NI], F32)

            eps_of = {}

            def emit_energy(kk, part):
                (base, width, tag, subs) = E_SPLITS[part]
                eps = ps_big.tile([P, width], F32, tag=tag, name=f"eps{part}")
                for (o0, ow) in subs:
                    nc.tensor.matmul(
                        eps[:, o0:o0 + ow],
                        q_t[:, kk * P:(kk + 1) * P],
                        k_t[:, base + o0:base + o0 + ow],
                        start=True, stop=True)
                eps_of.setdefault(kk, [None, None])[part] = eps

            # head: k_t tiles 0..2 + q_t tile 0 unlock energy(0) A; ACT
            # (idle until the first exp) takes k0/k2, DVE takes q0/k1.
            # Only q_t[:, 0:128] gates energy(0), so its bias lands in a
            # tiny op and the rest of tile 0 is deferred.
            psk_mms(*J_TILES[0], tag="engA", on_act=True)
            psq0 = _ps_tile("engB", "psq0")
            for c in range(2):
                nc.tensor.matmul(psq0[:CK, :512], wqk[:, c * P:c * P + CK],
                                 xt[:, c, 0:512],
                                 start=(c == 0), stop=(c == 1))
            nc.vector.tensor_scalar_add(q_t[:, 0:P], psq0[:CK, 0:P], qb)
            nc.vector.tensor_scalar_add(q_t[:, P:512], psq0[:CK, P:512], qb)
            psk_mms(*J_TILES[1], tag="rot0")
            psk_mms(*J_TILES[2], tag="rot1", on_act=True)
            emit_energy(0, 0)
            psk_mms(*J_TILES[3], tag="rot1")
            psk_mms(*J_TILES[4], tag="rot2")
            emit_energy(0, 1)
            psq_mms(*J_TILES[1], tag="rot0", defer=True)
            psq_mms(*J_TILES[2], tag="rot1", defer=True)
            psq_mms(*J_TILES[3], tag="rot2", defer=True)
            psq_mms(*J_TILES[4], tag="rot0", defer=True)
            # chunk 1's energy pre-emitted so PE runs it immediately after
            # the chunk-0 exps release the banks (keeps ACT gapless)
            emit_energy(1, 0)
            emit_energy(1, 1)

            # ---------------- V^T projection ----------------
            # fills PE idle in the head and early phase-1.  Slots are
            # assigned sequentially (bank0 gets chunks 0-5, ...) so each
            # rotation bank's first pass-2 segment is released as early as
            # possible; the first banks' bias-adds run on DVE (fast, early),
            # the rest on the otherwise-idle GPSIMD engine.
            with tc.high_priority(offset=-100000):
                for i in range(NI):
                    psv = ps_rot.tile([P, 512], F32, tag=f"rot{2 - i // 6}",
                                      name="psv")
                    for c in range(2):
                        nc.tensor.matmul(psv[:, :C],
                                         xt[:, c, i * P:(i + 1) * P],
                                         wv(c), start=(c == 0), stop=(c == 1))
                    nc.vector.tensor_tensor(vt[:, i], psv[:, :C], vb,
                                            ALU.add)

            # ---------------- pass 1 pipeline ----------------
            for k in range(NI):
                nc.scalar.activation(
                    out=expA[:, k, :], in_=eps_of[k][0][:],
                    func=AF.Exp, accum_out=s_half[:, k, 0:1])
                nc.scalar.activation(
                    out=expB[:, k, :], in_=eps_of[k][1][:],
                    func=AF.Exp)
                nc.vector.tensor_reduce(
                    s_half[:, k, 1:2], expB[:, k, :],
                    axis=AX.X, op=ALU.add)
                nc.vector.tensor_tensor(invs[:, k:k + 1], s_half[:, k, 0:1],
                                        s_half[:, k, 1:2], ALU.add)
                nc.vector.reciprocal(invs[:, k:k + 1], invs[:, k:k + 1])
                nc.vector.tensor_scalar_mul(vt[:, k], vt[:, k],
                                            invs[:, k:k + 1])
                if 2 <= k + 1 < NI:
                    emit_energy(k + 1, 0)
                    emit_energy(k + 1, 1)

            # ---------------- rotating pass-2 segments ----------------
            def exp_slice(i, j0, jw):
                if j0 + jw <= EA:
                    return expA[:, i, j0:j0 + jw]
                return expB[:, i, j0 - EA:j0 - EA + jw]

            def pair_mms(seg, i, start):
                # the two 256-wide regions share one PSUM bank; hardware
                # 'start' resets the whole bank, so the bank is pre-zeroed
                # once and every matmul accumulates (start=False)
                if start:
                    nc.vector.memset(seg[:], 0)
                for gi, g in enumerate(("G8", "G9")):
                    (ch, j0, jw) = GROUPS[g]
                    nc.tensor.matmul(
                        seg[:, gi * 256:gi * 256 + 256],
                        vt[:, i, ch * P:(ch + 1) * P],
                        exp_slice(i, j0, jw),
                        start=False, stop=False, skip_group_check=True)

            def group_mms(seg, g, i, start):
                if g == "PAIR":
                    pair_mms(seg, i, start)
                    return
                (ch, j0, jw) = GROUPS[g]
                nc.tensor.matmul(
                    seg[:, :jw],
                    vt[:, i, ch * P:(ch + 1) * P],
                    exp_slice(i, j0, jw),
                    start=start, stop=False, skip_group_check=True)

            partials = {}
            live_seg = {}
            for b, plan in enumerate(BANK_PLAN):
                for (g, i_lo, i_hi, dumped) in plan:
                    seg = ps_rot.tile([P, 512], F32, tag=f"rot{b}",
                                      name=f"seg_{g}")
                    for i in range(i_lo, i_hi + 1):
                        group_mms(seg, g, i, start=(i == i_lo))
                    if dumped:
                        # dump at normal priority: it gates the next
                        # segment's burst, so DVE must not defer it
                        part = big.tile([P, 512], F32, name=f"part_{g}")
                        nc.vector.tensor_copy(part[:], seg[:])
                        partials[g] = (part, i_lo, i_hi)
                    else:
                        live_seg[g] = (seg, i_lo, i_hi)

            # ---------------- phase 2 ----------------
            _queue_flip = [0]

            def emit_sbuf_store(g, ap):
                (ch, j0, jw) = GROUPS[g]
                _queue_flip[0] ^= 1
                eng = nc.sync if _queue_flip[0] else nc.scalar
                eng.dma_start(out_d[ch, :, j0:j0 + jw], ap)

            # live groups finish in place; bias-add, then store
            def finish_live(g):
                seg, i_lo, i_hi = live_seg[g]
                if g == "PAIR":
                    raise AssertionError("use finish_pair_region")
                else:
                    (ch, j0, jw) = GROUPS[g]
                    for i in range(i_hi + 1, NI - 1):
                        group_mms(seg, g, i, start=False)
                    nc.tensor.matmul(
                        seg[:, :jw],
                        vt[:, NI - 1, ch * P:(ch + 1) * P],
                        exp_slice(NI - 1, j0, jw),
                        start=False, stop=True, skip_group_check=True)
                    ot = work.tile([P, 512], F32, tag="out")
                    nc.vector.tensor_scalar_add(ot[:, :jw], seg[:, :jw],
                                                gbias[:, ch:ch + 1])
                    emit_sbuf_store(g, ot[:, :jw])

            # the PAIR's two regions close independently: G9 early in
            # phase 2, G8 last (its short chain is the kernel tail)
            def finish_pair_joint():
                seg, i_lo, i_hi = live_seg["PAIR"]
                for i in range(i_hi + 1, NI - 1):
                    pair_mms(seg, i, start=False)
                for gi, gg in enumerate(("G8", "G9")):
                    (ch, j0, jw) = GROUPS[gg]
                    nc.tensor.matmul(
                        seg[:, gi * 256:gi * 256 + 256],
                        vt[:, NI - 1, ch * P:(ch + 1) * P],
                        exp_slice(NI - 1, j0, jw),
                        start=False, stop=True, skip_group_check=True)
                for gi, gg in enumerate(("G8", "G9")):
                    (ch, j0, jw) = GROUPS[gg]
                    ot = work.tile([P, 256], F32, tag="out2")
                    nc.vector.tensor_scalar_add(
                        ot[:], seg[:, gi * 256:gi * 256 + 256],
                        gbias[:, ch:ch + 1])
                    eng = nc.sync if gi == 0 else nc.scalar
                    eng.dma_start(out_d[ch, :, j0:j0 + jw], ot[:])

            # dumped groups re-accumulate their remaining i-range in a fresh
            # slot, then combine (psum + gbias) + partial on DVE
            def finish_dumped(g, tag, pool, half=None, cols=None):
                part, i_lo, i_hi = partials[g]
                (ch, j0, jw) = GROUPS[g]
                if cols is not None:
                    c0, cw = cols
                else:
                    c0, cw = (0, jw) if half is None else (half * (jw // 2),
                                                          jw // 2)
                seg = pool.tile([P, 512], F32, tag=tag, name=f"fin_{g}")
                todo = list(range(0, i_lo)) + list(range(i_hi + 1, NI))
                for n, i in enumerate(todo):
                    nc.tensor.matmul(
                        seg[:, :cw],
                        vt[:, i, ch * P:(ch + 1) * P],
                        exp_slice(i, j0 + c0, cw),
                        start=(n == 0), stop=(n == len(todo) - 1),
                        skip_group_check=True)
                ot = work.tile([P, 512], F32, tag="out")
                nc.vector.scalar_tensor_tensor(
                    ot[:, :cw], seg[:, :cw], gbias[:, ch:ch + 1],
                    part[:, c0:c0 + cw], ALU.add, ALU.add)
                (ch_, j0_, jw_) = GROUPS[g]
                _queue_flip[0] ^= 1
                eng = nc.sync if _queue_flip[0] else nc.scalar
                eng.dma_start(out_d[ch_, :, j0_ + c0:j0_ + c0 + cw],
                              ot[:, :cw])

            # order: live 512 groups first (their stores overlap the rest),
            # dumped groups through freed energy + rot banks, PAIR last
            # (small, direct-from-PSUM stores).
            finish_live("G6")
            finish_live("G7")
            finish_dumped("G0", "engA", ps_big)
            finish_dumped("G1", "engB", ps_big)
            finish_dumped("G3", "rot0", ps_rot)
            finish_dumped("G4", "rot1", ps_rot)
            finish_dumped("G5", "engA", ps_big)
            finish_pair_joint()
            # last group: columns split across two banks so the first
            # half's store chain overlaps the second half's matmuls
            finish_dumped("G2", "engB", ps_big, cols=(0, 256))
            finish_dumped("G2", "engA", ps_big, cols=(256, 256))

    nc.compile()
    return nc


_NC_CACHE = []


def _get_nc():
    if not _NC_CACHE:
        _NC_CACHE.append(_build_nc())
    return _NC_CACHE[0]


def _prep_inputs(x, query_weight, query_bias, key_weight, key_bias,
                 value_weight, value_bias, gamma_weight, gamma_bias):
    bf16 = ml_dtypes.bfloat16
    x = np.asarray(x, np.float32).reshape(B, C, N)
    qw = np.asarray(query_weight, np.float32)[:, :, 0, 0]   # (64, 256)
    kw = np.asarray(key_weight, np.float32)[:, :, 0, 0]     # (64, 256)
    vw = np.asarray(value_weight, np.float32)[:, :, 0, 0]   # (256, 256)
    gw = np.asarray(gamma_weight, np.float32)[:, :, 0, 0]   # (256, 256)

    # wqk[p, c*128+m] = W_cat^T[c*128+p, m]  (W_cat = [Wq; Wk], (128, 256))
    wcat_t = np.concatenate([qw, kw], axis=0).T              # (256, 128)
    wqk = np.ascontiguousarray(
        wcat_t.reshape(2, P, P).transpose(1, 0, 2).reshape(P, 2 * P))

    # gamma folds into the V projection: W_comb = (Wg @ Wv)^T, bvg = Wg @ bv
    w_comb = (gw @ vw).T                                    # (c_in, o)
    wrest = np.ascontiguousarray(
        w_comb.reshape(2, P, C).transpose(1, 0, 2).reshape(P, 2 * C))
    bvg = gw @ np.asarray(value_bias, np.float32)

    wblob = np.zeros((P, 2 * P + C + 4), np.float32)
    wblob[:, 0:2 * P] = wqk
    wblob[0:CK, 2 * P] = np.asarray(query_bias, np.float32)
    wblob[0:CK, 2 * P + 1] = np.asarray(key_bias, np.float32)
    wblob[:, 2 * P + 2:2 * P + 4] = \
        np.asarray(gamma_bias, np.float32).reshape(2, P).T
    wblob[:, 2 * P + 4:] = bvg[None, :]

    base = {
        "wblob": wblob.astype(bf16),
        "wrest": wrest.astype(bf16),
    }
    in_maps = []
    for b in range(B):
        m = dict(base)
        m["x"] = x[b].reshape(2, P, N).astype(bf16)
        in_maps.append(m)
    return in_maps


def kernel(x, query_weight, query_bias, key_weight, key_bias,
           value_weight, value_bias, gamma_weight, gamma_bias, k):
    assert int(k) == C // CK
    in_maps = _prep_inputs(x, query_weight, query_bias, key_weight, key_bias,
                           value_weight, value_bias, gamma_weight, gamma_bias)
    nc = _get_nc()
    res = run_bass_kernel_spmd(nc, in_maps, core_ids=list(range(NCORES)))

    out = np.empty((B, C, H, W), np.float32)
    for b in range(B):
        out[b] = res.results[b]["out"].reshape(C, H, W)
    return out
